# revision 6
# baseline (speedup 1.0000x reference)
"""Trainium2 Bass kernel for the ConvolutionalKAN problem.

Math: out[b,o,y,x] = sum_{j,kk,l,m} phi_m(11*x[b,j,y+kk,x+l]) * coeff[o,j,kk,l,m]
with phi_m the cubic B-spline basis on uniform knots linspace(0,1,12):
6*phi_m(t) = relu(2-u)^3 - 4*relu(1-u)^3,  u = |t - (m+2)|, t = 11*x.

Identity used here (branch-free, relu-free until the end):
    6*phi = relu( 4*min(u-1,0)^3 - (u-2)^3 )
since for u<1 it equals 3u^3-6u^2+4 (>0), for 1<=u<2 it is (2-u)^3 (>=0),
and for u>=2 the argument is -(u-2)^3 < 0 so the outer relu zeroes it.

All elementwise math runs in fp16 (DVE 4x mode for tensor_scalar, 2x for
tensor_tensor); the scalar engine supplies |.| and the two squares with
fused per-partition bias. The conv is a VALID 3x3 over 512 channels
(4 contraction tiles of 128 = 2 basis fns x 64 cin), fp16 matmuls.

PE utilization trick: per (q,kk) the taps l=0 and l=1 are packed into one
M=128 stationary matrix (cols 0..63 -> l=0 out, 64..127 -> l=1 out) sharing
one 63-column stream; the l=2 taps accumulate as M=64 matmuls into the SAME
PSUM bank, column-aligned with the l=0 partials. One 24-matmul chain per
8-row output group, then a single cross-partition add fixes up l=1's
one-column shift: out = ps[0:64,:,0:62] + ps[64:128,:,1:63].

Sharding: data-parallel over batch, 2 images per core on 8 cores.
"""

import os
import sys

import numpy as np

for _p in ("/root/.axon_site/_ro/trn_rl_repo", "/opt/trn_rl_repo"):
    if os.path.isdir(_p) and _p not in sys.path:
        sys.path.append(_p)

B_FULL = 16
N_CORES = 8
B_SHARD = B_FULL // N_CORES
CIN = 64
COUT = 64
H = 64
W = 64
KS = 3
NB = 8
NQ = 4            # contraction tiles: 128 = 2 basis fns x 64 cin
HO = H - KS + 1   # 62
WO = W - KS + 1   # 62
NPIX = H * W      # 4096
CHUNK = 2048      # elementwise chain chunk (free-dim) per pass
GROUPS = [(0, 8), (8, 8), (16, 8), (24, 8), (32, 8), (40, 8), (48, 8), (56, 6)]


def _fold_weights(coeff: np.ndarray):
    """coeff [O,J,KS,KS,NB] -> wpair [NQ,KS,128,128] f16, wsing [NQ,KS,128,64] f16.

    wpair[q,kk,k,l*64+o] = coeff[o,j(k),kk,l,m(q,k)]/6 for l in {0,1};
    wsing[q,kk,k,o] likewise for l=2. k<64: m=2q,j=k; k>=64: m=2q+1,j=k-64.
    """
    a = (coeff.astype(np.float64) / 6.0).transpose(4, 1, 2, 3, 0)  # [m,j,kk,l,o]
    a = a.reshape(NQ, 2, CIN, KS, KS, COUT).transpose(0, 3, 1, 2, 4, 5)
    # a: [q, kk, mhalf, j, l, o]
    wpair = a[:, :, :, :, 0:2, :].reshape(NQ, KS, 128, 2 * COUT)
    wsing = a[:, :, :, :, 2, :].reshape(NQ, KS, 128, COUT)
    # flatten to [128, NQ*KS*cols] (k on partitions) so the DMA is 2-D
    wpair = np.ascontiguousarray(
        wpair.transpose(2, 0, 1, 3).reshape(128, NQ * KS * 2 * COUT)
    ).astype(np.float16)
    wsing = np.ascontiguousarray(
        wsing.transpose(2, 0, 1, 3).reshape(128, NQ * KS * COUT)
    ).astype(np.float16)
    return wpair, wsing


def _build_bass():
    import concourse.bacc as bacc
    import concourse.mybir as mybir
    import concourse.tile as tile

    f32 = mybir.dt.float32
    f16 = mybir.dt.float16
    alu = mybir.AluOpType
    AF = mybir.ActivationFunctionType

    nc = bacc.Bacc("TRN2", target_bir_lowering=False, debug=False,
                   num_devices=N_CORES)
    x_d = nc.dram_tensor("x", [B_SHARD, CIN, H, W], f32, kind="ExternalInput").ap()
    wp_d = nc.dram_tensor("wpair", [128, NQ * KS * 2 * COUT], f16,
                          kind="ExternalInput").ap()
    ws_d = nc.dram_tensor("wsing", [128, NQ * KS * COUT], f16,
                          kind="ExternalInput").ap()
    b_d = nc.dram_tensor("btbl", [128, NQ + 2], f32, kind="ExternalInput").ap()
    out_d = nc.dram_tensor("out", [B_SHARD, COUT, HO, WO], f32,
                           kind="ExternalOutput").ap()

    with tile.TileContext(nc) as tc:
        from contextlib import ExitStack

        with ExitStack() as ctx:
            cpool = ctx.enter_context(tc.tile_pool(name="const", bufs=1))
            xpool = ctx.enter_context(tc.tile_pool(name="x", bufs=2))
            gpool = ctx.enter_context(tc.tile_pool(name="g", bufs=2))
            rpool = ctx.enter_context(tc.tile_pool(name="chain", bufs=2))
            opool = ctx.enter_context(tc.tile_pool(name="o", bufs=4))
            ppool = ctx.enter_context(
                tc.tile_pool(name="ps", bufs=3, space="PSUM"))

            bt = cpool.tile([128, NQ + 2], f32)
            nc.sync.dma_start(bt[:], b_d[:])
            wp = cpool.tile([128, NQ * KS * 2 * COUT], f16, tag="wp")
            nc.sync.dma_start(wp[:], wp_d[:])
            ws = cpool.tile([128, NQ * KS * COUT], f16, tag="ws")
            nc.sync.dma_start(ws[:], ws_d[:])

            for b in range(B_SHARD):
                xt = xpool.tile([128, NPIX], f32)
                src = x_d[b, :, :, :]
                xv = xt[:].rearrange("p (r c) -> p r c", c=W)
                nc.gpsimd.dma_start(xv[0:64], src)
                nc.gpsimd.dma_start(xv[64:128], src)

                gts = []
                for q in range(NQ):
                    g = gpool.tile([128, NPIX], f16, tag=f"g{q}")
                    for c0 in range(0, NPIX, CHUNK):
                        cs = slice(c0, c0 + CHUNK)
                        u = rpool.tile([128, CHUNK], f16, tag="u")
                        na = rpool.tile([128, CHUNK], f16, tag="na")
                        sa = rpool.tile([128, CHUNK], f16, tag="sa")
                        t = rpool.tile([128, CHUNK], f16, tag="t")
                        nb = rpool.tile([128, CHUNK], f16, tag="nb")
                        sb = rpool.tile([128, CHUNK], f16, tag="sb")
                        t1 = rpool.tile([128, CHUNK], f16, tag="t1")
                        # u = |11x - (m+2)|   (f32 in, fp16 out)
                        nc.scalar.activation(u[:], xt[:, cs], AF.Abs,
                                             bias=bt[:, q:q + 1], scale=11.0)
                        # na = min(u-2, 0)  (clamp makes t self-gating at u>=2)
                        nc.vector.tensor_scalar(na[:], u[:], 2.0, 0.0,
                                                alu.subtract, alu.min)
                        # sa = (u-2)^2
                        nc.scalar.activation(sa[:], u[:], AF.Square,
                                             bias=bt[:, NQ:NQ + 1], scale=1.0)
                        # t = min(u-2,0)^3  (sa*na; sa wrong for u>2 but na=0 there)
                        nc.vector.tensor_tensor(t[:], sa[:], na[:], alu.mult)
                        # nb = min(u-1, 0)
                        nc.vector.tensor_scalar(nb[:], u[:], 1.0, 0.0,
                                                alu.subtract, alu.min)
                        # sb = (2*nb)^2 = 4*(1-u1)^2
                        nc.scalar.activation(sb[:], nb[:], AF.Square,
                                             bias=bt[:, NQ + 1:NQ + 2],
                                             scale=2.0)
                        # t1 = 4*min(u-1,0)^3
                        nc.vector.tensor_tensor(t1[:], sb[:], nb[:], alu.mult)
                        # g = t1 - t = 6*phi exactly (both clamped)
                        nc.vector.tensor_tensor(g[:, cs], t1[:], t[:],
                                                alu.subtract)
                    gts.append(g)

                gvs = [g[:].rearrange("p (r c) -> p r c", c=W) for g in gts]
                for gi, (y0, nr) in enumerate(GROUPS):
                    ps = ppool.tile([128, 8, 63], f32, tag="ps")
                    n_mm = NQ * KS * 2
                    i_mm = 0
                    for q in range(NQ):
                        for kk in range(KS):
                            # taps (l=0, l=1) packed in M=128, one stream
                            lhsT = wp[:, (q * KS + kk) * 2 * COUT:
                                      (q * KS + kk + 1) * 2 * COUT]
                            rhs = gvs[q][:, y0 + kk:y0 + kk + nr, 0:63]
                            nc.tensor.matmul(ps[:, :nr, :], lhsT, rhs,
                                             start=(i_mm == 0), stop=False,
                                             skip_group_check=True)
                            i_mm += 1
                            # tap l=2, M=64, aligned with the l=0 partials
                            lhsT2 = ws[:, (q * KS + kk) * COUT:
                                       (q * KS + kk + 1) * COUT]
                            rhs2 = gvs[q][:, y0 + kk:y0 + kk + nr, 2:64]
                            nc.tensor.matmul(ps[0:64, :nr, 0:62], lhsT2, rhs2,
                                             start=False, stop=(i_mm == n_mm - 1),
                                             skip_group_check=True)
                            i_mm += 1
                    ot = opool.tile([64, 8, WO], f32)
                    tmp = opool.tile([64, 8, WO], f32, tag="tmp")
                    # stage l1 partials (shifted one column) in SBUF, then add
                    if gi % 2 == 0:
                        nc.scalar.activation(tmp[:, :nr, :],
                                             ps[64:128, :nr, 1:63], AF.Identity,
                                             bias=bt[0:64, NQ + 1:NQ + 2],
                                             scale=1.0)
                    else:
                        nc.vector.tensor_copy(tmp[:, :nr, :],
                                              ps[64:128, :nr, 1:63])
                    nc.vector.tensor_tensor(ot[:, :nr, :], ps[0:64, :nr, 0:62],
                                            tmp[:, :nr, :], alu.add)
                    nc.sync.dma_start(out_d[b, :, y0:y0 + nr, :], ot[:, :nr, :])

    nc.compile()
    return nc


def _maybe_install_profile_shim():
    """Allow trace=True/BASS_TRACE under axon even though this image lacks
    antenv.axon_hooks; degrade silently if anything is missing."""
    import types

    if "antenv.axon_hooks" in sys.modules:
        return
    try:
        from trn_agent_boot.trn_boot import _ntff_profile_via_ctypes

        hook = _ntff_profile_via_ctypes("/opt/axon/libaxon_pjrt.so")
        if hook is None:
            return
        mod = types.ModuleType("antenv.axon_hooks")
        mod.get_axon_ntff_profile_hook = lambda: hook
        mod.set_axon_ntff_profile_hook = lambda h: None
        sys.modules["antenv.axon_hooks"] = mod
        from concourse import bass_utils

        bass_utils.upload_artifacts = lambda tmpdir: f"local:{tmpdir}"
    except Exception:
        pass


_LAST_RESULTS = None


def kernel(x: np.ndarray, coeff: np.ndarray) -> np.ndarray:
    global _LAST_RESULTS
    from concourse import bass_utils

    _maybe_install_profile_shim()

    x = np.ascontiguousarray(np.asarray(x), dtype=np.float32)
    coeff = np.asarray(coeff)
    assert x.shape == (B_FULL, CIN, H, W), x.shape

    if os.environ.get("KAN_TRACE"):
        # The NTFF profile hook needs the axon PJRT client initialized by a
        # real device execute before axon_start_nrt_profile, else rc=-1.
        import jax.numpy as jnp

        (jnp.zeros((8,), jnp.float32) + 1.0).block_until_ready()

    wpair, wsing = _fold_weights(coeff)
    btbl = np.zeros((128, NQ + 2), dtype=np.float32)
    for p in range(128):
        for q in range(NQ):
            m = 2 * q + (1 if p >= 64 else 0)
            btbl[p, q] = -float(m + 2)
    btbl[:, NQ] = -2.0
    btbl[:, NQ + 1] = 0.0

    nc = _build_bass()

    in_maps = []
    for i in range(N_CORES):
        in_maps.append({
            "x": np.ascontiguousarray(x[i * B_SHARD:(i + 1) * B_SHARD]),
            "wpair": wpair,
            "wsing": wsing,
            "btbl": btbl,
        })

    res = bass_utils.run_bass_kernel_spmd(
        nc, in_maps, core_ids=list(range(N_CORES)),
        trace=bool(os.environ.get("KAN_TRACE")),
    )
    _LAST_RESULTS = res

    out = np.concatenate([res.results[i]["out"] for i in range(N_CORES)], axis=0)
    return out.astype(np.float32, copy=False)


# revision 8
# speedup vs baseline: 1.3047x; 1.3047x over previous
"""Trainium2 Bass kernel for the ConvolutionalKAN problem.

Math: out[b,o,y,x] = sum_{j,kk,l,m} phi_m(11*x[b,j,y+kk,x+l]) * coeff[o,j,kk,l,m]
with phi_m the cubic B-spline basis on uniform knots linspace(0,1,12):
6*phi_m(t) = relu(2-u)^3 - 4*relu(1-u)^3,  u = |t - (m+2)|, t = 11*x.

Identity used here (branch-free, relu-free until the end):
    6*phi = relu( 4*min(u-1,0)^3 - (u-2)^3 )
since for u<1 it equals 3u^3-6u^2+4 (>0), for 1<=u<2 it is (2-u)^3 (>=0),
and for u>=2 the argument is -(u-2)^3 < 0 so the outer relu zeroes it.

All elementwise math runs in fp16 (DVE 4x mode for tensor_scalar, 2x for
tensor_tensor); the scalar engine supplies |.| and the two squares with
fused per-partition bias. The conv is a VALID 3x3 over 512 channels
(4 contraction tiles of 128 = 2 basis fns x 64 cin), fp16 matmuls.

PE utilization trick: per (q,kk) the taps l=0 and l=1 are packed into one
M=128 stationary matrix (cols 0..63 -> l=0 out, 64..127 -> l=1 out) sharing
one 63-column stream; the l=2 taps accumulate as M=64 matmuls into the SAME
PSUM bank, column-aligned with the l=0 partials. One 24-matmul chain per
8-row output group, then a single cross-partition add fixes up l=1's
one-column shift: out = ps[0:64,:,0:62] + ps[64:128,:,1:63].

Sharding: data-parallel over batch, 2 images per core on 8 cores.
"""

import os
import sys

import numpy as np

for _p in ("/root/.axon_site/_ro/trn_rl_repo", "/opt/trn_rl_repo"):
    if os.path.isdir(_p) and _p not in sys.path:
        sys.path.append(_p)

B_FULL = 16
N_CORES = 8
B_SHARD = B_FULL // N_CORES
CIN = 64
COUT = 64
H = 64
W = 64
KS = 3
NB = 8
NQ = 4            # contraction tiles: 128 = 2 basis fns x 64 cin
HO = H - KS + 1   # 62
WO = W - KS + 1   # 62
NPIX = H * W      # 4096
CHUNK = 1024      # elementwise chain chunk (free-dim) per pass
GROUPS = [(0, 8), (8, 8), (16, 8), (24, 8), (32, 8), (40, 8), (48, 8), (56, 6)]


def _fold_weights(coeff: np.ndarray):
    """coeff [O,J,KS,KS,NB] -> wpair [NQ,KS,128,128] f16, wsing [NQ,KS,128,64] f16.

    wpair[q,kk,k,l*64+o] = coeff[o,j(k),kk,l,m(q,k)]/6 for l in {0,1};
    wsing[q,kk,k,o] likewise for l=2. k<64: m=2q,j=k; k>=64: m=2q+1,j=k-64.
    """
    a = (coeff.astype(np.float64) / 6.0).transpose(4, 1, 2, 3, 0)  # [m,j,kk,l,o]
    a = a.reshape(NQ, 2, CIN, KS, KS, COUT).transpose(0, 3, 1, 2, 4, 5)
    # a: [q, kk, mhalf, j, l, o]
    wpair = a[:, :, :, :, 0:2, :].reshape(NQ, KS, 128, 2 * COUT)
    wsing = a[:, :, :, :, 2, :].reshape(NQ, KS, 128, COUT)
    # flatten to [128, NQ*KS*cols] (k on partitions) so the DMA is 2-D
    wpair = np.ascontiguousarray(
        wpair.transpose(2, 0, 1, 3).reshape(128, NQ * KS * 2 * COUT)
    ).astype(np.float16)
    wsing = np.ascontiguousarray(
        wsing.transpose(2, 0, 1, 3).reshape(128, NQ * KS * COUT)
    ).astype(np.float16)
    return wpair, wsing


def _build_bass():
    import concourse.bacc as bacc
    import concourse.mybir as mybir
    import concourse.tile as tile

    f32 = mybir.dt.float32
    f16 = mybir.dt.float16
    alu = mybir.AluOpType
    AF = mybir.ActivationFunctionType

    nc = bacc.Bacc("TRN2", target_bir_lowering=False, debug=False,
                   num_devices=N_CORES)
    x_d = nc.dram_tensor("x", [B_SHARD, CIN, H, W], f32, kind="ExternalInput").ap()
    wp_d = nc.dram_tensor("wpair", [128, NQ * KS * 2 * COUT], f16,
                          kind="ExternalInput").ap()
    ws_d = nc.dram_tensor("wsing", [128, NQ * KS * COUT], f16,
                          kind="ExternalInput").ap()
    b_d = nc.dram_tensor("btbl", [128, NQ + 2], f32, kind="ExternalInput").ap()
    out_d = nc.dram_tensor("out", [B_SHARD, COUT, HO, WO], f32,
                           kind="ExternalOutput").ap()

    with tile.TileContext(nc) as tc:
        from contextlib import ExitStack

        with ExitStack() as ctx:
            cpool = ctx.enter_context(tc.tile_pool(name="const", bufs=1))
            xpool = ctx.enter_context(tc.tile_pool(name="x", bufs=2))
            gpool = ctx.enter_context(tc.tile_pool(name="g", bufs=2))
            rpool = ctx.enter_context(tc.tile_pool(name="chain", bufs=2))
            opool = ctx.enter_context(tc.tile_pool(name="o", bufs=8))
            ppool = ctx.enter_context(
                tc.tile_pool(name="ps", bufs=6, space="PSUM"))

            bt = cpool.tile([128, NQ + 2], f32)
            nc.sync.dma_start(bt[:], b_d[:])
            wp = cpool.tile([128, NQ * KS * 2 * COUT], f16, tag="wp")
            nc.sync.dma_start(wp[:], wp_d[:])
            ws = cpool.tile([128, NQ * KS * COUT], f16, tag="ws")
            nc.sync.dma_start(ws[:], ws_d[:])

            for b in range(B_SHARD):
                xt = xpool.tile([128, NPIX], f32)
                src = x_d[b, :, :, :]
                xv = xt[:].rearrange("p (r c) -> p r c", c=W)
                nc.gpsimd.dma_start(xv[0:64], src)
                nc.gpsimd.dma_start(xv[64:128], src)

                gts = [gpool.tile([128, NPIX], f16, tag=f"g{q}",
                                  name=f"g{q}") for q in range(NQ)]
                for c0 in range(0, NPIX, CHUNK):
                    cs = slice(c0, c0 + CHUNK)
                    for q in range(NQ):
                        g = gts[q]
                        u = rpool.tile([128, CHUNK], f16, tag="u")
                        na = rpool.tile([128, CHUNK], f16, tag="na")
                        sa = rpool.tile([128, CHUNK], f16, tag="sa")
                        t = rpool.tile([128, CHUNK], f16, tag="t")
                        nb = rpool.tile([128, CHUNK], f16, tag="nb")
                        sb = rpool.tile([128, CHUNK], f16, tag="sb")
                        t1 = rpool.tile([128, CHUNK], f16, tag="t1")
                        # u = |11x - (m+2)|   (f32 in, fp16 out)
                        nc.scalar.activation(u[:], xt[:, cs], AF.Abs,
                                             bias=bt[:, q:q + 1], scale=11.0)
                        # na = min(u-2, 0)  (clamp makes t self-gating at u>=2)
                        nc.vector.tensor_scalar(na[:], u[:], 2.0, 0.0,
                                                alu.subtract, alu.min)
                        # sa = (u-2)^2
                        nc.scalar.activation(sa[:], u[:], AF.Square,
                                             bias=bt[:, NQ:NQ + 1], scale=1.0)
                        # t = min(u-2,0)^3  (sa*na; sa wrong for u>2 but na=0 there)
                        nc.vector.tensor_tensor(t[:], sa[:], na[:], alu.mult)
                        # nb = min(u-1, 0)
                        nc.vector.tensor_scalar(nb[:], u[:], 1.0, 0.0,
                                                alu.subtract, alu.min)
                        # sb = (2*nb)^2 = 4*(1-u1)^2
                        nc.scalar.activation(sb[:], nb[:], AF.Square,
                                             bias=bt[:, NQ + 1:NQ + 2],
                                             scale=2.0)
                        # t1 = 4*min(u-1,0)^3
                        nc.vector.tensor_tensor(t1[:], sb[:], nb[:], alu.mult)
                        # g = t1 - t = 6*phi exactly (both clamped)
                        nc.vector.tensor_tensor(g[:, cs], t1[:], t[:],
                                                alu.subtract)

                gvs = [g[:].rearrange("p (r c) -> p r c", c=W) for g in gts]
                for gi, (y0, nr) in enumerate(GROUPS):
                    ps = ppool.tile([128, 8, 63], f32, tag="ps")
                    n_mm = NQ * KS * 2
                    i_mm = 0
                    for q in range(NQ):
                        for kk in range(KS):
                            # taps (l=0, l=1) packed in M=128, one stream
                            lhsT = wp[:, (q * KS + kk) * 2 * COUT:
                                      (q * KS + kk + 1) * 2 * COUT]
                            rhs = gvs[q][:, y0 + kk:y0 + kk + nr, 0:63]
                            nc.tensor.matmul(ps[:, :nr, :], lhsT, rhs,
                                             start=(i_mm == 0), stop=False,
                                             skip_group_check=True)
                            i_mm += 1
                            # tap l=2, M=64, aligned with the l=0 partials
                            lhsT2 = ws[:, (q * KS + kk) * COUT:
                                       (q * KS + kk + 1) * COUT]
                            rhs2 = gvs[q][:, y0 + kk:y0 + kk + nr, 2:64]
                            nc.tensor.matmul(ps[0:64, :nr, 0:62], lhsT2, rhs2,
                                             start=False, stop=(i_mm == n_mm - 1),
                                             skip_group_check=True)
                            i_mm += 1
                    ot = opool.tile([64, 8, WO], f32)
                    tmp = opool.tile([64, 8, WO], f32, tag="tmp")
                    # stage l1 partials (shifted one column) in SBUF, then add
                    if gi % 2 == 0:
                        nc.scalar.activation(tmp[:, :nr, :],
                                             ps[64:128, :nr, 1:63], AF.Identity,
                                             bias=bt[0:64, NQ + 1:NQ + 2],
                                             scale=1.0)
                    else:
                        nc.vector.tensor_copy(tmp[:, :nr, :],
                                              ps[64:128, :nr, 1:63])
                    nc.vector.tensor_tensor(ot[:, :nr, :], ps[0:64, :nr, 0:62],
                                            tmp[:, :nr, :], alu.add)
                    nc.sync.dma_start(out_d[b, :, y0:y0 + nr, :], ot[:, :nr, :])

    nc.compile()
    return nc


def _maybe_install_profile_shim():
    """Allow trace=True/BASS_TRACE under axon even though this image lacks
    antenv.axon_hooks; degrade silently if anything is missing."""
    import types

    if "antenv.axon_hooks" in sys.modules:
        return
    try:
        from trn_agent_boot.trn_boot import _ntff_profile_via_ctypes

        hook = _ntff_profile_via_ctypes("/opt/axon/libaxon_pjrt.so")
        if hook is None:
            return
        mod = types.ModuleType("antenv.axon_hooks")
        mod.get_axon_ntff_profile_hook = lambda: hook
        mod.set_axon_ntff_profile_hook = lambda h: None
        sys.modules["antenv.axon_hooks"] = mod
        from concourse import bass_utils

        bass_utils.upload_artifacts = lambda tmpdir: f"local:{tmpdir}"
    except Exception:
        pass


_LAST_RESULTS = None


def kernel(x: np.ndarray, coeff: np.ndarray) -> np.ndarray:
    global _LAST_RESULTS
    from concourse import bass_utils

    _maybe_install_profile_shim()

    x = np.ascontiguousarray(np.asarray(x), dtype=np.float32)
    coeff = np.asarray(coeff)
    assert x.shape == (B_FULL, CIN, H, W), x.shape

    if os.environ.get("KAN_TRACE"):
        # The NTFF profile hook needs the axon PJRT client initialized by a
        # real device execute before axon_start_nrt_profile, else rc=-1.
        import jax.numpy as jnp

        (jnp.zeros((8,), jnp.float32) + 1.0).block_until_ready()

    wpair, wsing = _fold_weights(coeff)
    btbl = np.zeros((128, NQ + 2), dtype=np.float32)
    for p in range(128):
        for q in range(NQ):
            m = 2 * q + (1 if p >= 64 else 0)
            btbl[p, q] = -float(m + 2)
    btbl[:, NQ] = -2.0
    btbl[:, NQ + 1] = 0.0

    nc = _build_bass()

    in_maps = []
    for i in range(N_CORES):
        in_maps.append({
            "x": np.ascontiguousarray(x[i * B_SHARD:(i + 1) * B_SHARD]),
            "wpair": wpair,
            "wsing": wsing,
            "btbl": btbl,
        })

    res = bass_utils.run_bass_kernel_spmd(
        nc, in_maps, core_ids=list(range(N_CORES)),
        trace=bool(os.environ.get("KAN_TRACE")),
    )
    _LAST_RESULTS = res

    out = np.concatenate([res.results[i]["out"] for i in range(N_CORES)], axis=0)
    return out.astype(np.float32, copy=False)
